# revision 18
# baseline (speedup 1.0000x reference)
"""EmmaAttention EMA-merge kernel for 8 Trainium2 NeuronCores.

Computation (per node n, head h):
    beta  = clip(1 - inv_w * agg_n[n], 0, 1)
    max_m = max(max_a, his_m)
    p     = exp(his_m - max_m) * beta
    q     = exp(max_a - max_m)
    t     = max(p + q, 1.0)
    out[n,h,:] = his_x[n,h,:] * (p/t) + x[n,h,:] * (q/t)

Pure elementwise over N -> shard N across the 8 cores, no communication.

Per-core layout: Nc = 25000 nodes on P = 125 partitions, 200 nodes per
partition (node = partition*200 + g).  Per-(node,head) scalars p/t, q/t are
precomputed once into SBUF ([125, 1600]), then the [125, G*512] main-loop
tiles multiply against them via stride-0 broadcast APs over D=64.
"""

import numpy as np

N, H, D = 200000, 8, 64
HD = H * D
NCORES = 8
NC_SHARD = N // NCORES  # 25000 nodes per core
P = 125                 # SBUF partitions used (25000 = 125 * 200)
NPP = NC_SHARD // P     # 200 nodes per partition
G = 4                   # nodes-per-partition per main-loop tile
NT = NPP // G           # 50 main-loop tiles
FD = G * HD             # 5120 f32 free-dim elements per tile
SH = G * H              # 80 (node,head) scalars per tile per partition

_CACHE = {}


def _build_program():
    from concourse import mybir, tile, bacc
    from concourse.bass import ts

    # 2x the default SWDGE descriptor carveout: the ring otherwise fills
    # after ~8 queued DMAs and caps how far descriptor generation can run
    # ahead of the SDMA engines during the ramp.
    nc = bacc.Bacc(trn_type="TRN2", dynamic_dma_scratch_size=32768)
    f32 = mybir.dt.float32
    bf16 = mybir.dt.bfloat16

    x = nc.dram_tensor("x", (NC_SHARD, H, D), f32, kind="ExternalInput")
    max_a = nc.dram_tensor("max_a", (NC_SHARD, H), f32, kind="ExternalInput")
    his_x = nc.dram_tensor("his_x", (NC_SHARD, H, D), f32, kind="ExternalInput")
    his_m = nc.dram_tensor("his_m", (NC_SHARD, H), f32, kind="ExternalInput")
    agg_n = nc.dram_tensor("agg_n", (NC_SHARD,), f32, kind="ExternalInput")
    inv_w = nc.dram_tensor("inv_w", (1,), f32, kind="ExternalInput")
    out = nc.dram_tensor("out", (NC_SHARD, H, D), f32, kind="ExternalOutput")

    x3 = x[:].rearrange("(p g) h d -> p g (h d)", p=P)     # [125, 200, 512]
    hx3 = his_x[:].rearrange("(p g) h d -> p g (h d)", p=P)
    o3 = out[:].rearrange("(p g) h d -> p g (h d)", p=P)
    ma2 = max_a[:].rearrange("(p g) h -> p (g h)", p=P)    # [125, 1600]
    hm2 = his_m[:].rearrange("(p g) h -> p (g h)", p=P)
    an2 = agg_n[:].rearrange("(p g) -> p g", p=P)          # [125, 200]

    Alu = mybir.AluOpType
    Act = mybir.ActivationFunctionType

    with tile.TileContext(nc) as tc:
        with tc.tile_pool(name="persist", bufs=1) as pp:
            p_t = pp.tile((P, NPP * H), f32)
            q_t = pp.tile((P, NPP * H), f32)
            # bf16 copies of the p/t, q/t scalars for the bf16 main loop
            pb_t = pp.tile((P, NPP * H), bf16)
            qb_t = pp.tile((P, NPP * H), bf16)

            # The scratch pool stays open for the whole kernel: if it
            # closed, the main-loop pool would reuse its SBUF addresses and
            # the first big loads would inherit a WAR dependency on all of
            # phase A (costs ~40us of pipeline ramp).
            with (
                tc.tile_pool(name="scratch", bufs=1) as sp,
                tc.tile_pool(name="bigx", bufs=12) as bpx,
                tc.tile_pool(name="bigh", bufs=18) as bph,
            ):
                # Small loads go on the same SWDGE queue as the bulk
                # traffic, BEFORE it: the queue is FIFO, so they land in the
                # first microseconds.  (On the HWDGE queue they fight the
                # bulk stream for SDMA engines 64-68 and can land ~80us
                # late, stalling all of phase A and then the whole pipe.)
                ma_t = sp.tile((P, NPP * H), f32)
                nc.gpsimd.dma_start(ma_t[:], ma2)
                hm_t = sp.tile((P, NPP * H), f32)
                nc.gpsimd.dma_start(hm_t[:], hm2)
                an_t = sp.tile((P, NPP), f32)
                nc.gpsimd.dma_start(an_t[:], an2)
                iw_t = sp.tile((P, 1), f32)
                nc.gpsimd.dma_start(iw_t[:], inv_w[:].to_broadcast((P, 1)))

                mm_t = sp.tile((P, NPP * H), f32)
                bt_t = sp.tile((P, NPP), f32)
                niw_t = sp.tile((P, 1), f32)
                zero_t = sp.tile((P, 1), f32)
                one_t = sp.tile((P, 1), f32)

                # Const [P,1] tiles, built on ScalarE.  All phase-A DVE ops
                # below are 2-src tensor_tensor (1x mode): single-src
                # tensor_scalar ops can engage the DVE 2-port perf mode,
                # which locks GpSimd out of SBUF while SWDGE descriptor
                # generation for the concurrent bulk DMAs needs it.
                nc.scalar.mul(zero_t[:], iw_t[:], 0.0)
                nc.scalar.activation(one_t[:], zero_t[:], Act.Copy, bias=1.0)
                # p/t and q/t scalars, [125, 1600] (g-major, h-minor).
                # Computed in column chunks so the first main-loop tile's
                # multiplies can start after ~1/4 of phase A instead of
                # waiting for the whole serial DVE chain (incl. the
                # 8-cycle/elem reciprocal).
                nc.scalar.mul(niw_t[:], iw_t[:], -1.0)
                PC = 4
                CW = NPP * H // PC   # scalar columns per chunk
                GW = NPP // PC       # node columns per chunk
                for c in range(PC):
                    cs = ts(c, CW)
                    gs = ts(c, GW)
                    ma_c, hm_c, mm_c = ma_t[:, cs], hm_t[:, cs], mm_t[:, cs]
                    p_c, q_c = p_t[:, cs], q_t[:, cs]
                    an_c, bt_c = an_t[:, gs], bt_t[:, gs]
                    nc.vector.tensor_max(mm_c, ma_c, hm_c)
                    nc.vector.tensor_sub(hm_c, hm_c, mm_c)
                    nc.vector.tensor_sub(ma_c, ma_c, mm_c)
                    nc.scalar.activation(p_c, hm_c, Act.Exp)
                    nc.scalar.activation(q_c, ma_c, Act.Exp)
                    # beta = clip(1 - inv_w*agg_n, 0, 1); p *= beta over h
                    nc.vector.tensor_mul(
                        bt_c, an_c, niw_t[:].to_broadcast((P, GW))
                    )
                    nc.vector.tensor_add(bt_c, bt_c, one_t[:].to_broadcast((P, GW)))
                    nc.vector.tensor_max(bt_c, bt_c, zero_t[:].to_broadcast((P, GW)))
                    nc.vector.tensor_tensor(
                        bt_c, bt_c, one_t[:].to_broadcast((P, GW)), Alu.min
                    )
                    p3 = p_c.rearrange("p (g h) -> p g h", h=H)
                    nc.vector.tensor_mul(
                        p3, p3, bt_c[:, :, None].to_broadcast((P, GW, H))
                    )
                    # r = 1 / max(p + q, 1)
                    nc.vector.tensor_add(mm_c, p_c, q_c)
                    nc.vector.tensor_max(mm_c, mm_c, one_t[:].to_broadcast((P, CW)))
                    # t >= 1 always, so the fast approx (no 0/denorm/inf
                    # handling, ~18 good bits) is safe and ~5x cheaper than
                    # the 8-cycle/elem exact reciprocal.
                    nc.vector.reciprocal_approx_fast(mm_c, mm_c)
                    nc.vector.tensor_mul(p_c, p_c, mm_c)
                    nc.vector.tensor_mul(q_c, q_c, mm_c)
                    nc.vector.tensor_copy(pb_t[:, cs], p_c)
                    nc.vector.tensor_copy(qb_t[:, cs], q_c)

                # main loop: out = his_x * p + x * q, p/q broadcast over
                # D.  All bulk DMAs ride the gpsimd SWDGE queue: it sprays
                # across all 16 SDMA engines, while the HWDGE rows are
                # slower AND poison SWDGE throughput when mixed in
                # (measured: loads 186->145 GB/s, mix 237->176 GB/s).
                # Small G is deliberate: per-engine packet rate is ~flat
                # (~18 GB/s while busy) but engine OCCUPANCY rises as
                # packets shrink (measured mix: G=10 237, G=5 265, G=2
                # 284 GB/s) - 4 KiB rows keep all 16 engines ~100% busy.
                # Stores are delayed by DL iterations.  The store's
                # descriptor emission on Q7 waits for the DVE result; with a
                # 1-iteration delay that wait fires every iteration and caps
                # load emission at ~1 tile ahead of DVE, starving the SDMA
                # engines (measured: Q7 sem-wait bound, engines ~73% busy).
                # With DL=8 the waited-on tile is long done, so Q7 streams
                # descriptors without ever blocking.
                # The whole main loop runs in bf16: SWDGE casts f32->bf16 on
                # the loads and bf16->f32 on the store, in flight.  The SDMA
                # engine's per-packet cost tracks the SBUF-side bytes, so
                # halving them nearly doubles effective HBM bandwidth
                # (measured single-core mix: 284 -> 374 GB/s HBM-side).
                DL = 4
                pend = []
                for t in range(NT):
                    x_t = bpx.tile((P, FD), bf16)
                    nc.gpsimd.dma_start(x_t[:], x3[:, ts(t, G), :])
                    h_t = bph.tile((P, FD), bf16)
                    nc.gpsimd.dma_start(h_t[:], hx3[:, ts(t, G), :])
                    if len(pend) >= DL:
                        ts_, prev = pend.pop(0)
                        nc.gpsimd.dma_start(o3[:, ts(ts_, G), :], prev[:])

                    h3 = h_t[:].rearrange("p (s d) -> p s d", d=D)
                    xx3 = x_t[:].rearrange("p (s d) -> p s d", d=D)
                    pb = pb_t[:, ts(t, SH)][:, :, None].to_broadcast((P, SH, D))
                    qb = qb_t[:, ts(t, SH)][:, :, None].to_broadcast((P, SH, D))
                    nc.vector.tensor_mul(h3, h3, pb)
                    nc.vector.tensor_mul(xx3, xx3, qb)
                    nc.vector.tensor_add(h_t[:], h_t[:], x_t[:])
                    pend.append((t, h_t))
                for ts_, prev in pend:
                    nc.gpsimd.dma_start(o3[:, ts(ts_, G), :], prev[:])

    nc.finalize()
    return nc


def _get_program():
    if "nc" not in _CACHE:
        _CACHE["nc"] = _build_program()
    return _CACHE["nc"]


def _make_in_maps(x, max_a, his_x, his_m, agg_n, inv_w):
    x = np.ascontiguousarray(x, dtype=np.float32)
    max_a = np.ascontiguousarray(max_a, dtype=np.float32)
    his_x = np.ascontiguousarray(his_x, dtype=np.float32)
    his_m = np.ascontiguousarray(his_m, dtype=np.float32)
    agg_n = np.ascontiguousarray(agg_n, dtype=np.float32)
    inv_w = np.ascontiguousarray(inv_w, dtype=np.float32)
    in_maps = []
    for c in range(NCORES):
        s = slice(c * NC_SHARD, (c + 1) * NC_SHARD)
        in_maps.append(
            {
                "x": x[s],
                "max_a": max_a[s],
                "his_x": his_x[s],
                "his_m": his_m[s],
                "agg_n": agg_n[s],
                "inv_w": inv_w,
            }
        )
    return in_maps


def kernel_run(x, max_a, his_x, his_m, agg_n, inv_w, **run_kwargs):
    """Run on HW; returns (full_output, BassKernelResults)."""
    from concourse.bass_utils import run_bass_kernel_spmd

    nc = _get_program()
    in_maps = _make_in_maps(x, max_a, his_x, his_m, agg_n, inv_w)
    res = run_bass_kernel_spmd(nc, in_maps, core_ids=list(range(NCORES)), **run_kwargs)
    full = np.concatenate([res.results[c]["out"] for c in range(NCORES)], axis=0)
    return full, res


def kernel(x, max_a, his_x, his_m, agg_n, inv_w):
    full, _ = kernel_run(x, max_a, his_x, his_m, agg_n, inv_w)
    return full



# revision 21
# speedup vs baseline: 1.0805x; 1.0805x over previous
"""EmmaAttention EMA-merge kernel for 8 Trainium2 NeuronCores.

Computation (per node n, head h):
    beta  = clip(1 - inv_w * agg_n[n], 0, 1)
    max_m = max(max_a, his_m)
    p     = exp(his_m - max_m) * beta
    q     = exp(max_a - max_m)
    t     = max(p + q, 1.0)
    out[n,h,:] = his_x[n,h,:] * (p/t) + x[n,h,:] * (q/t)

Pure elementwise over N -> shard N across the 8 cores, no communication.

Per-core layout: Nc = 25000 nodes on P = 125 partitions, 200 nodes per
partition (node = partition*200 + g).  Per-(node,head) scalars p/t, q/t are
precomputed once into SBUF ([125, 1600]), then the [125, G*512] main-loop
tiles multiply against them via stride-0 broadcast APs over D=64.
"""

import numpy as np

N, H, D = 200000, 8, 64
HD = H * D
NCORES = 8
NC_SHARD = N // NCORES  # 25000 nodes per core
P = 125                 # SBUF partitions used (25000 = 125 * 200)
NPP = NC_SHARD // P     # 200 nodes per partition
G = 4                   # nodes-per-partition per main-loop tile
NT = NPP // G           # 50 main-loop tiles
FD = G * HD             # 5120 f32 free-dim elements per tile
SH = G * H              # 80 (node,head) scalars per tile per partition

_CACHE = {}


def _build_program():
    from concourse import mybir, tile, bacc
    from concourse.bass import ts

    nc = bacc.Bacc(trn_type="TRN2")
    f32 = mybir.dt.float32
    bf16 = mybir.dt.bfloat16

    x = nc.dram_tensor("x", (NC_SHARD, H, D), f32, kind="ExternalInput")
    max_a = nc.dram_tensor("max_a", (NC_SHARD, H), f32, kind="ExternalInput")
    his_x = nc.dram_tensor("his_x", (NC_SHARD, H, D), f32, kind="ExternalInput")
    his_m = nc.dram_tensor("his_m", (NC_SHARD, H), f32, kind="ExternalInput")
    agg_n = nc.dram_tensor("agg_n", (NC_SHARD,), f32, kind="ExternalInput")
    inv_w = nc.dram_tensor("inv_w", (1,), f32, kind="ExternalInput")
    out = nc.dram_tensor("out", (NC_SHARD, H, D), f32, kind="ExternalOutput")

    x3 = x[:].rearrange("(p g) h d -> p g (h d)", p=P)     # [125, 200, 512]
    hx3 = his_x[:].rearrange("(p g) h d -> p g (h d)", p=P)
    o3 = out[:].rearrange("(p g) h d -> p g (h d)", p=P)
    ma2 = max_a[:].rearrange("(p g) h -> p (g h)", p=P)    # [125, 1600]
    hm2 = his_m[:].rearrange("(p g) h -> p (g h)", p=P)
    an2 = agg_n[:].rearrange("(p g) -> p g", p=P)          # [125, 200]

    Alu = mybir.AluOpType
    Act = mybir.ActivationFunctionType

    with tile.TileContext(nc) as tc:
        with tc.tile_pool(name="persist", bufs=1) as pp:
            p_t = pp.tile((P, NPP * H), f32)
            q_t = pp.tile((P, NPP * H), f32)
            # bf16 copies of the p/t, q/t scalars for the bf16 main loop
            pb_t = pp.tile((P, NPP * H), bf16)
            qb_t = pp.tile((P, NPP * H), bf16)

            # The scratch pool stays open for the whole kernel: if it
            # closed, the main-loop pool would reuse its SBUF addresses and
            # the first big loads would inherit a WAR dependency on all of
            # phase A (costs ~40us of pipeline ramp).
            with (
                tc.tile_pool(name="scratch", bufs=1) as sp,
                tc.tile_pool(name="bigx", bufs=12) as bpx,
                tc.tile_pool(name="bigh", bufs=18) as bph,
            ):
                # Small loads go on the same SWDGE queue as the bulk
                # traffic, BEFORE it: the queue is FIFO, so they land in the
                # first microseconds.  (On the HWDGE queue they fight the
                # bulk stream for SDMA engines 64-68 and can land ~80us
                # late, stalling all of phase A and then the whole pipe.)
                ma_t = sp.tile((P, NPP * H), f32)
                nc.gpsimd.dma_start(ma_t[:], ma2)
                hm_t = sp.tile((P, NPP * H), f32)
                nc.gpsimd.dma_start(hm_t[:], hm2)
                an_t = sp.tile((P, NPP), f32)
                nc.gpsimd.dma_start(an_t[:], an2)
                iw_t = sp.tile((P, 1), f32)
                nc.gpsimd.dma_start(iw_t[:], inv_w[:].to_broadcast((P, 1)))

                mm_t = sp.tile((P, NPP * H), f32)
                bt_t = sp.tile((P, NPP), f32)
                niw_t = sp.tile((P, 1), f32)
                zero_t = sp.tile((P, 1), f32)
                one_t = sp.tile((P, 1), f32)

                # Const [P,1] tiles, built on ScalarE.  All phase-A DVE ops
                # below are 2-src tensor_tensor (1x mode): single-src
                # tensor_scalar ops can engage the DVE 2-port perf mode,
                # which locks GpSimd out of SBUF while SWDGE descriptor
                # generation for the concurrent bulk DMAs needs it.
                nc.scalar.mul(zero_t[:], iw_t[:], 0.0)
                nc.scalar.activation(one_t[:], zero_t[:], Act.Copy, bias=1.0)
                # p/t and q/t scalars, [125, 1600] (g-major, h-minor).
                # Computed in column chunks so the first main-loop tile's
                # multiplies can start after ~1/4 of phase A instead of
                # waiting for the whole serial DVE chain (incl. the
                # 8-cycle/elem reciprocal).
                nc.scalar.mul(niw_t[:], iw_t[:], -1.0)
                PC = 4
                CW = NPP * H // PC   # scalar columns per chunk
                GW = NPP // PC       # node columns per chunk

                def phase_a_chunk(c):
                    cs = ts(c, CW)
                    gs = ts(c, GW)
                    ma_c, hm_c, mm_c = ma_t[:, cs], hm_t[:, cs], mm_t[:, cs]
                    p_c, q_c = p_t[:, cs], q_t[:, cs]
                    an_c, bt_c = an_t[:, gs], bt_t[:, gs]
                    nc.vector.tensor_max(mm_c, ma_c, hm_c)
                    nc.vector.tensor_sub(hm_c, hm_c, mm_c)
                    nc.vector.tensor_sub(ma_c, ma_c, mm_c)
                    nc.scalar.activation(p_c, hm_c, Act.Exp)
                    nc.scalar.activation(q_c, ma_c, Act.Exp)
                    # beta = clip(1 - inv_w*agg_n, 0, 1); p *= beta over h
                    nc.vector.tensor_mul(
                        bt_c, an_c, niw_t[:].to_broadcast((P, GW))
                    )
                    nc.vector.tensor_add(bt_c, bt_c, one_t[:].to_broadcast((P, GW)))
                    nc.vector.tensor_max(bt_c, bt_c, zero_t[:].to_broadcast((P, GW)))
                    nc.vector.tensor_tensor(
                        bt_c, bt_c, one_t[:].to_broadcast((P, GW)), Alu.min
                    )
                    p3 = p_c.rearrange("p (g h) -> p g h", h=H)
                    nc.vector.tensor_mul(
                        p3, p3, bt_c[:, :, None].to_broadcast((P, GW, H))
                    )
                    # r = 1 / max(p + q, 1)
                    nc.vector.tensor_add(mm_c, p_c, q_c)
                    nc.vector.tensor_max(mm_c, mm_c, one_t[:].to_broadcast((P, CW)))
                    # t >= 1 always, so the fast approx (no 0/denorm/inf
                    # handling, ~18 good bits) is safe and ~5x cheaper than
                    # the 8-cycle/elem exact reciprocal.
                    nc.vector.reciprocal_approx_fast(mm_c, mm_c)
                    nc.vector.tensor_mul(p_c, p_c, mm_c)
                    nc.vector.tensor_mul(q_c, q_c, mm_c)
                    nc.vector.tensor_copy(pb_t[:, cs], p_c)
                    nc.vector.tensor_copy(qb_t[:, cs], q_c)

                # DVE executes in program order, so emitting all of phase A
                # up front would delay the first main-loop multiply by the
                # whole serial chunk chain (~25us).  Instead each chunk is
                # emitted just before the first tile that reads its columns.
                next_chunk = 0

                def emit_ready_chunks(t):
                    nonlocal next_chunk
                    while next_chunk < PC and ((t + 1) * SH - 1) // CW >= next_chunk:
                        phase_a_chunk(next_chunk)
                        next_chunk += 1

                # main loop: out = his_x * p + x * q, p/q broadcast over
                # D.  All bulk DMAs ride the gpsimd SWDGE queue: it sprays
                # across all 16 SDMA engines, while the HWDGE rows are
                # slower AND poison SWDGE throughput when mixed in
                # (measured: loads 186->145 GB/s, mix 237->176 GB/s).
                # Small G is deliberate: per-engine packet rate is ~flat
                # (~18 GB/s while busy) but engine OCCUPANCY rises as
                # packets shrink (measured mix: G=10 237, G=5 265, G=2
                # 284 GB/s) - 4 KiB rows keep all 16 engines ~100% busy.
                # Stores are delayed by DL iterations.  The store's
                # descriptor emission on Q7 waits for the DVE result; with a
                # 1-iteration delay that wait fires every iteration and caps
                # load emission at ~1 tile ahead of DVE, starving the SDMA
                # engines (measured: Q7 sem-wait bound, engines ~73% busy).
                # With DL=8 the waited-on tile is long done, so Q7 streams
                # descriptors without ever blocking.
                # The whole main loop runs in bf16: SWDGE casts f32->bf16 on
                # the loads and bf16->f32 on the store, in flight.  The SDMA
                # engine's per-packet cost tracks the SBUF-side bytes, so
                # halving them nearly doubles effective HBM bandwidth
                # (measured single-core mix: 284 -> 374 GB/s HBM-side).
                DL = 4
                pend = []
                for t in range(NT):
                    x_t = bpx.tile((P, FD), bf16)
                    nc.gpsimd.dma_start(x_t[:], x3[:, ts(t, G), :])
                    h_t = bph.tile((P, FD), bf16)
                    nc.gpsimd.dma_start(h_t[:], hx3[:, ts(t, G), :])
                    if len(pend) >= DL:
                        ts_, prev = pend.pop(0)
                        nc.gpsimd.dma_start(o3[:, ts(ts_, G), :], prev[:])

                    emit_ready_chunks(t)
                    h3 = h_t[:].rearrange("p (s d) -> p s d", d=D)
                    xx3 = x_t[:].rearrange("p (s d) -> p s d", d=D)
                    pb = pb_t[:, ts(t, SH)][:, :, None].to_broadcast((P, SH, D))
                    qb = qb_t[:, ts(t, SH)][:, :, None].to_broadcast((P, SH, D))
                    nc.vector.tensor_mul(h3, h3, pb)
                    nc.vector.tensor_mul(xx3, xx3, qb)
                    nc.vector.tensor_add(h_t[:], h_t[:], x_t[:])
                    pend.append((t, h_t))
                for ts_, prev in pend:
                    nc.gpsimd.dma_start(o3[:, ts(ts_, G), :], prev[:])

    nc.finalize()
    return nc


def _get_program():
    if "nc" not in _CACHE:
        _CACHE["nc"] = _build_program()
    return _CACHE["nc"]


def _make_in_maps(x, max_a, his_x, his_m, agg_n, inv_w):
    x = np.ascontiguousarray(x, dtype=np.float32)
    max_a = np.ascontiguousarray(max_a, dtype=np.float32)
    his_x = np.ascontiguousarray(his_x, dtype=np.float32)
    his_m = np.ascontiguousarray(his_m, dtype=np.float32)
    agg_n = np.ascontiguousarray(agg_n, dtype=np.float32)
    inv_w = np.ascontiguousarray(inv_w, dtype=np.float32)
    in_maps = []
    for c in range(NCORES):
        s = slice(c * NC_SHARD, (c + 1) * NC_SHARD)
        in_maps.append(
            {
                "x": x[s],
                "max_a": max_a[s],
                "his_x": his_x[s],
                "his_m": his_m[s],
                "agg_n": agg_n[s],
                "inv_w": inv_w,
            }
        )
    return in_maps


def kernel_run(x, max_a, his_x, his_m, agg_n, inv_w, **run_kwargs):
    """Run on HW; returns (full_output, BassKernelResults)."""
    from concourse.bass_utils import run_bass_kernel_spmd

    nc = _get_program()
    in_maps = _make_in_maps(x, max_a, his_x, his_m, agg_n, inv_w)
    res = run_bass_kernel_spmd(nc, in_maps, core_ids=list(range(NCORES)), **run_kwargs)
    full = np.concatenate([res.results[c]["out"] for c in range(NCORES)], axis=0)
    return full, res


def kernel(x, max_a, his_x, his_m, agg_n, inv_w):
    full, _ = kernel_run(x, max_a, his_x, his_m, agg_n, inv_w)
    return full



# revision 23
# speedup vs baseline: 1.2605x; 1.1665x over previous
"""EmmaAttention EMA-merge kernel for 8 Trainium2 NeuronCores.

Computation (per node n, head h):
    beta  = clip(1 - inv_w * agg_n[n], 0, 1)
    max_m = max(max_a, his_m)
    p     = exp(his_m - max_m) * beta
    q     = exp(max_a - max_m)
    t     = max(p + q, 1.0)
    out[n,h,:] = his_x[n,h,:] * (p/t) + x[n,h,:] * (q/t)

Pure elementwise over N -> shard N across the 8 cores, no communication.

Per-core layout: Nc = 25000 nodes on P = 125 partitions, 200 nodes per
partition (node = partition*200 + g).  Per-(node,head) scalars p/t, q/t are
precomputed once into SBUF ([125, 1600]), then the [125, G*512] main-loop
tiles multiply against them via stride-0 broadcast APs over D=64.
"""

import numpy as np

N, H, D = 200000, 8, 64
HD = H * D
NCORES = 8
NC_SHARD = N // NCORES  # 25000 nodes per core
P = 125                 # SBUF partitions used (25000 = 125 * 200)
NPP = NC_SHARD // P     # 200 nodes per partition
G = 4                   # nodes-per-partition per main-loop tile
NT = NPP // G           # 50 main-loop tiles
FD = G * HD             # 5120 f32 free-dim elements per tile
SH = G * H              # 80 (node,head) scalars per tile per partition

_CACHE = {}


def _build_program():
    from concourse import mybir, tile, bacc
    from concourse.bass import ts

    nc = bacc.Bacc(trn_type="TRN2")
    f32 = mybir.dt.float32
    bf16 = mybir.dt.bfloat16

    x = nc.dram_tensor("x", (NC_SHARD, H, D), f32, kind="ExternalInput")
    max_a = nc.dram_tensor("max_a", (NC_SHARD, H), f32, kind="ExternalInput")
    his_x = nc.dram_tensor("his_x", (NC_SHARD, H, D), f32, kind="ExternalInput")
    his_m = nc.dram_tensor("his_m", (NC_SHARD, H), f32, kind="ExternalInput")
    agg_n = nc.dram_tensor("agg_n", (NC_SHARD,), f32, kind="ExternalInput")
    inv_w = nc.dram_tensor("inv_w", (1,), f32, kind="ExternalInput")
    out = nc.dram_tensor("out", (NC_SHARD, H, D), f32, kind="ExternalOutput")

    x3 = x[:].rearrange("(p g) h d -> p g (h d)", p=P)     # [125, 200, 512]
    hx3 = his_x[:].rearrange("(p g) h d -> p g (h d)", p=P)
    o3 = out[:].rearrange("(p g) h d -> p g (h d)", p=P)
    ma2 = max_a[:].rearrange("(p g) h -> p (g h)", p=P)    # [125, 1600]
    hm2 = his_m[:].rearrange("(p g) h -> p (g h)", p=P)
    an2 = agg_n[:].rearrange("(p g) -> p g", p=P)          # [125, 200]

    Alu = mybir.AluOpType
    Act = mybir.ActivationFunctionType

    with tile.TileContext(nc) as tc:
        with tc.tile_pool(name="persist", bufs=1) as pp:
            p_t = pp.tile((P, NPP * H), f32)
            q_t = pp.tile((P, NPP * H), f32)
            # bf16 copies of the p/t, q/t scalars for the bf16 main loop
            pb_t = pp.tile((P, NPP * H), bf16)
            qb_t = pp.tile((P, NPP * H), bf16)

            # The scratch pool stays open for the whole kernel: if it
            # closed, the main-loop pool would reuse its SBUF addresses and
            # the first big loads would inherit a WAR dependency on all of
            # phase A (costs ~40us of pipeline ramp).
            with (
                tc.tile_pool(name="scratch", bufs=1) as sp,
                tc.tile_pool(name="bigx", bufs=12) as bpx,
                tc.tile_pool(name="bigh", bufs=18) as bph,
            ):
                # Small loads go on the same SWDGE queue as the bulk
                # traffic, BEFORE it: the queue is FIFO, so they land in the
                # first microseconds.  (On the HWDGE queue they fight the
                # bulk stream for SDMA engines 64-68 and can land ~80us
                # late, stalling all of phase A and then the whole pipe.)
                ma_t = sp.tile((P, NPP * H), f32)
                nc.gpsimd.dma_start(ma_t[:], ma2)
                hm_t = sp.tile((P, NPP * H), f32)
                nc.gpsimd.dma_start(hm_t[:], hm2)
                an_t = sp.tile((P, NPP), f32)
                nc.gpsimd.dma_start(an_t[:], an2)
                iw_t = sp.tile((P, 1), f32)
                nc.gpsimd.dma_start(iw_t[:], inv_w[:].to_broadcast((P, 1)))

                mm_t = sp.tile((P, NPP * H), f32)
                bt_t = sp.tile((P, NPP), f32)
                niw_t = sp.tile((P, 1), f32)
                zero_t = sp.tile((P, 1), f32)
                one_t = sp.tile((P, 1), f32)

                # Const [P,1] tiles, built on ScalarE.  All phase-A DVE ops
                # below are 2-src tensor_tensor (1x mode): single-src
                # tensor_scalar ops can engage the DVE 2-port perf mode,
                # which locks GpSimd out of SBUF while SWDGE descriptor
                # generation for the concurrent bulk DMAs needs it.
                nc.scalar.mul(zero_t[:], iw_t[:], 0.0)
                nc.scalar.activation(one_t[:], zero_t[:], Act.Copy, bias=1.0)
                # p/t and q/t scalars, [125, 1600] (g-major, h-minor).
                # Computed in column chunks so the first main-loop tile's
                # multiplies can start after ~1/4 of phase A instead of
                # waiting for the whole serial DVE chain (incl. the
                # 8-cycle/elem reciprocal).
                nc.scalar.mul(niw_t[:], iw_t[:], -1.0)
                PC = 4
                CW = NPP * H // PC   # scalar columns per chunk
                GW = NPP // PC       # node columns per chunk

                def phase_a_chunk(c):
                    cs = ts(c, CW)
                    gs = ts(c, GW)
                    ma_c, hm_c, mm_c = ma_t[:, cs], hm_t[:, cs], mm_t[:, cs]
                    p_c, q_c = p_t[:, cs], q_t[:, cs]
                    an_c, bt_c = an_t[:, gs], bt_t[:, gs]
                    nc.vector.tensor_max(mm_c, ma_c, hm_c)
                    nc.vector.tensor_sub(hm_c, hm_c, mm_c)
                    nc.vector.tensor_sub(ma_c, ma_c, mm_c)
                    nc.scalar.activation(p_c, hm_c, Act.Exp)
                    nc.scalar.activation(q_c, ma_c, Act.Exp)
                    # beta = clip(1 - inv_w*agg_n, 0, 1); p *= beta over h
                    nc.vector.tensor_mul(
                        bt_c, an_c, niw_t[:].to_broadcast((P, GW))
                    )
                    nc.vector.tensor_add(bt_c, bt_c, one_t[:].to_broadcast((P, GW)))
                    nc.vector.tensor_max(bt_c, bt_c, zero_t[:].to_broadcast((P, GW)))
                    nc.vector.tensor_tensor(
                        bt_c, bt_c, one_t[:].to_broadcast((P, GW)), Alu.min
                    )
                    p3 = p_c.rearrange("p (g h) -> p g h", h=H)
                    nc.vector.tensor_mul(
                        p3, p3, bt_c[:, :, None].to_broadcast((P, GW, H))
                    )
                    # r = 1 / max(p + q, 1)
                    nc.vector.tensor_add(mm_c, p_c, q_c)
                    nc.vector.tensor_max(mm_c, mm_c, one_t[:].to_broadcast((P, CW)))
                    # t >= 1 always, so the fast approx (no 0/denorm/inf
                    # handling, ~18 good bits) is safe and ~5x cheaper than
                    # the 8-cycle/elem exact reciprocal.
                    nc.vector.reciprocal_approx_fast(mm_c, mm_c)
                    nc.vector.tensor_mul(p_c, p_c, mm_c)
                    nc.vector.tensor_mul(q_c, q_c, mm_c)
                    nc.vector.tensor_copy(pb_t[:, cs], p_c)
                    nc.vector.tensor_copy(qb_t[:, cs], q_c)

                for c in range(PC):
                    phase_a_chunk(c)

                # main loop: out = his_x * p + x * q, p/q broadcast over
                # D.  All bulk DMAs ride the gpsimd SWDGE queue: it sprays
                # across all 16 SDMA engines, while the HWDGE rows are
                # slower AND poison SWDGE throughput when mixed in
                # (measured: loads 186->145 GB/s, mix 237->176 GB/s).
                # Small G is deliberate: per-engine packet rate is ~flat
                # (~18 GB/s while busy) but engine OCCUPANCY rises as
                # packets shrink (measured mix: G=10 237, G=5 265, G=2
                # 284 GB/s) - 4 KiB rows keep all 16 engines ~100% busy.
                # Stores are delayed by DL iterations.  The store's
                # descriptor emission on Q7 waits for the DVE result; with a
                # 1-iteration delay that wait fires every iteration and caps
                # load emission at ~1 tile ahead of DVE, starving the SDMA
                # engines (measured: Q7 sem-wait bound, engines ~73% busy).
                # With DL=8 the waited-on tile is long done, so Q7 streams
                # descriptors without ever blocking.
                # The whole main loop runs in bf16: SWDGE casts f32->bf16 on
                # the loads and bf16->f32 on the store, in flight.  The SDMA
                # engine's per-packet cost tracks the SBUF-side bytes, so
                # halving them nearly doubles effective HBM bandwidth
                # (measured single-core mix: 284 -> 374 GB/s HBM-side).
                DL = 4
                pend = []
                for t in range(NT):
                    x_t = bpx.tile((P, FD), bf16)
                    nc.gpsimd.dma_start(x_t[:], x3[:, ts(t, G), :])
                    h_t = bph.tile((P, FD), bf16)
                    nc.gpsimd.dma_start(h_t[:], hx3[:, ts(t, G), :])
                    if len(pend) >= DL:
                        ts_, prev = pend.pop(0)
                        nc.gpsimd.dma_start(o3[:, ts(ts_, G), :], prev[:])

                    h3 = h_t[:].rearrange("p (s d) -> p s d", d=D)
                    xx3 = x_t[:].rearrange("p (s d) -> p s d", d=D)
                    pb = pb_t[:, ts(t, SH)][:, :, None].to_broadcast((P, SH, D))
                    qb = qb_t[:, ts(t, SH)][:, :, None].to_broadcast((P, SH, D))
                    nc.vector.tensor_mul(h3, h3, pb)
                    nc.vector.tensor_mul(xx3, xx3, qb)
                    nc.vector.tensor_add(h_t[:], h_t[:], x_t[:])
                    pend.append((t, h_t))
                for ts_, prev in pend:
                    nc.gpsimd.dma_start(o3[:, ts(ts_, G), :], prev[:])

    nc.finalize()
    return nc


def _get_program():
    if "nc" not in _CACHE:
        _CACHE["nc"] = _build_program()
    return _CACHE["nc"]


def _make_in_maps(x, max_a, his_x, his_m, agg_n, inv_w):
    x = np.ascontiguousarray(x, dtype=np.float32)
    max_a = np.ascontiguousarray(max_a, dtype=np.float32)
    his_x = np.ascontiguousarray(his_x, dtype=np.float32)
    his_m = np.ascontiguousarray(his_m, dtype=np.float32)
    agg_n = np.ascontiguousarray(agg_n, dtype=np.float32)
    inv_w = np.ascontiguousarray(inv_w, dtype=np.float32)
    in_maps = []
    for c in range(NCORES):
        s = slice(c * NC_SHARD, (c + 1) * NC_SHARD)
        in_maps.append(
            {
                "x": x[s],
                "max_a": max_a[s],
                "his_x": his_x[s],
                "his_m": his_m[s],
                "agg_n": agg_n[s],
                "inv_w": inv_w,
            }
        )
    return in_maps


def kernel_run(x, max_a, his_x, his_m, agg_n, inv_w, **run_kwargs):
    """Run on HW; returns (full_output, BassKernelResults)."""
    from concourse.bass_utils import run_bass_kernel_spmd

    nc = _get_program()
    in_maps = _make_in_maps(x, max_a, his_x, his_m, agg_n, inv_w)
    res = run_bass_kernel_spmd(nc, in_maps, core_ids=list(range(NCORES)), **run_kwargs)
    full = np.concatenate([res.results[c]["out"] for c in range(NCORES)], axis=0)
    return full, res


def kernel(x, max_a, his_x, his_m, agg_n, inv_w):
    full, _ = kernel_run(x, max_a, his_x, his_m, agg_n, inv_w)
    return full

